# revision 116
# baseline (speedup 1.0000x reference)
"""nn_MultiHeadAttention_84954453115654 — Trainium2 Bass kernel, 8 NeuronCores.

Sharding: data-parallel over batch (2) x head-pair-parallel (4 groups of 2
heads).  Core c handles batch b = c//4 and embed rows [128*(c%4), +128)
(= heads 2*(c%4), 2*(c%4)+1).  Host sums the 4 out_proj partials per batch
and adds bo (the row-parallel all-reduce, done at gather time).

Key idea: the masks are inputs, so the host COMPACTS the time axis before
launch.  Only unmasked query columns (Tq ~ half of 2048) and unmasked key
columns (Tk ~ half) are shipped/computed; value_mask is pre-applied to the
compacted xv.  Masked query columns of the output are exactly bo (reference:
softmax scores row zeroed -> av col 0 -> out col = bias), so the host
scatters computed columns back and fills the rest with bo.  This halves DMA
and PE work and quarters the softmax-exp ACT work vs. dense.

Weight prep happens on host (weights are inputs): weight-standardize, fold
the per-head centering (I - M2) into Wq/Wk/Wv so the projection matmul
directly yields diff = p - mean_head(p), transpose into lhsT layout, bf16.

Per-core device program:
  - q/k/v projections (bf16 matmuls, K-chained over 4 tiles of 128) with
    fused per-head LayerNorm: var via block-mean matmul of diff^2, then
    out = (diff * cscale) * reciprocal(sqrt(var + eps)) (1/SCALE folded
    into q's cscale)
  - scores transposed per (tk-tile i, head, tq-tile): S^T[tk, tq] =
    kn_i^T @ qn chunk; softmax exp on ACT with a per-partition bias (-80
    on compact-pad keys, else 0); no row-max subtraction (post-LN scores
    are O(0.1) so exp cannot overflow)
  - av[65, jw] += vaugT_i @ exp accumulated over i in PSUM; vaug carries a
    ones column so row 64 is the softmax denominator; av matmuls are
    spread between exp units to keep the ACT stream dense
  - lazy LN for k/v: kn/vn keep the centered diff; the per-column 1/sd
    (computed pre-transposed via tiny dsq^T @ m2vec matmuls) folds into
    the exp scale operand (k) and the vaug transpose drain (v)
  - normalize per tq-chunk: denominator rows batched into one tile, one
    reciprocal, PE-broadcast per row, avn = avsb * bcast; out_proj
    partial woT[:, m] @ avn drains bf16 and DMAs per (chunk, m) so the
    first chunk's output overlaps the last exps
"""

import os
import sys
import contextlib
import functools

for _p in ("/root/.axon_site/_ro/trn_rl_repo", "/opt/trn_rl_repo"):
    if os.path.isdir(_p) and _p not in sys.path:
        sys.path.append(_p)

import numpy as np
import ml_dtypes

import concourse.bass as bass
import concourse.mybir as mybir
import concourse.tile as tile
from concourse import bass_utils

B, E, T, H = 2, 512, 2048, 8
DH = E // H            # 64
HPC = 2                # heads per core
G = 4                  # cores per batch group
NCORES = 8
NK = E // 128          # 4 contraction tiles for the projections
EPS = 1e-5
SCALE = float(E // H ** 0.5)   # 181.0
BIG = 80.0
FP = mybir.dt.float32
FR = mybir.dt.float32r
BF = mybir.dt.bfloat16
AF = mybir.ActivationFunctionType
OP = mybir.AluOpType
BF_NP = np.dtype(ml_dtypes.bfloat16)


def _split_multiwaits(nc):
    """Split multi-wait instructions (Tile's tail drain) into single-wait
    EventSemaphore chains; this container's walrus encodes only one sync
    wait per instruction."""
    import bass_rust

    n_new = 0
    for f in nc.m.functions:
        for bb in f.blocks:
            out = []
            changed = False
            for ins in bb.instructions:
                si = ins.sync_info
                if si is not None and si.on_wait is not None and len(si.on_wait) > 1:
                    waits = list(si.on_wait)
                    for w in waits[:-1]:
                        ev = bass_rust.InstEventSemaphore(
                            name=f"MWFIX-{n_new}", ins=[], outs=[]
                        )
                        n_new += 1
                        ev.engine = ins.engine
                        ev.sync_info = bass_rust.SyncInfo(on_wait=[w], on_update=[])
                        out.append(ev)
                    ins.sync_info = bass_rust.SyncInfo(
                        on_wait=[waits[-1]], on_update=list(si.on_update or [])
                    )
                    changed = True
                out.append(ins)
            if changed:
                bb.instructions = out
    return n_new


def _chunks(total, step=512):
    out = []
    off = 0
    while off < total:
        out.append((off, min(step, total - off)))
        off += step
    return out


def _jt_split(Tq):
    """scores/exp tile widths: one tile if it fits 2 PSUM banks, else
    (512, rest) so the first exp only needs the first qn chunk."""
    if Tq <= 1024:
        return [(0, Tq)]
    assert Tq <= 1536, f"Tq={Tq} too large for the 2-tile scores layout"
    return [(0, 512), (512, Tq - 512)]


def _emit(nc, tc, dram, flags, dbg, Tq, Tk):
    v = nc.vector
    sc = nc.scalar
    te = nc.tensor
    gp = nc.gpsimd
    sy = nc.sync

    NIK = Tk // 128          # tk tiles
    JQ = _chunks(Tq)         # tq chunks (512-wide, last may be ragged)
    JK = _chunks(Tk)
    JT = _jt_split(Tq)
    use_bias = flags["use_bias"]
    use_affine = flags["use_affine"]

    stack = contextlib.ExitStack()

    consts = stack.enter_context(tc.tile_pool(name="consts", bufs=1))
    xcp = stack.enter_context(tc.tile_pool(name="xc", bufs=2))
    nbuf = stack.enter_context(tc.tile_pool(name="named", bufs=1))
    ntp = stack.enter_context(
        tc.tile_pool(name="nt", bufs=(Tk // 128) * HPC * len(_jt_split(Tq)) + 2)
    )
    sdp = stack.enter_context(tc.tile_pool(name="sd", bufs=3))
    dnp = stack.enter_context(tc.tile_pool(name="dn", bufs=1))

    # ---- packed constant loads -----------------------------------------
    # wqk: [wq | wk | m2b] bf16 (needed before the first exp);
    # wvo: [wv | woT] bf16 (queued after the early x chunks);
    # cpackr: [identr | onesP] fp32r; mbias separate fp32.
    cpackr = consts.tile([128, 128 + DH], FR, tag="cpackr")
    sy.dma_start(cpackr[:], dram["cpackr"])
    identr = cpackr[:, 0:128]
    onesP = cpackr[:, 128 : 128 + DH]  # all-ones rows (base-aligned lhsT)
    mbias = consts.tile([128, NIK], FP, tag="mbias")
    sy.dma_start(mbias[:], dram["mbias"])
    wqk = consts.tile([128, 2 * E + 130], BF, tag="wqk")
    sy.dma_start(wqk[:], dram["wqk"])
    wvo = consts.tile([128, 2 * E], BF, tag="wvo")
    wT = {"wq": wqk[:, 0:E], "wk": wqk[:, E : 2 * E], "wv": wvo[:, 0:E]}
    m2b = wqk[:, 2 * E : 2 * E + 128]
    m2vec = wqk[:, 2 * E + 128 : 2 * E + 130]  # per-head 1/64 column pair
    woT = wvo[:, E : 2 * E]
    epst = consts.tile([128, 1], FP, tag="eps")
    v.memset(epst[:], EPS)

    bias_tiles = {}
    if use_bias:
        for bn in ("bcq", "bck", "bcv"):
            bt = consts.tile([128, 1], FP, tag=bn, name=bn)
            sy.dma_start(bt[:], dram[bn])
            bias_tiles[bn] = bt
    ge_tiles = {}
    if use_affine:
        for gn in ("geq", "beq", "gek", "bek", "gev", "bev"):
            gt = consts.tile([128, 1], FP, tag=gn, name=gn)
            sy.dma_start(gt[:], dram[gn])
            ge_tiles[gn] = gt

    # ---- x loads -------------------------------------------------------
    # Host packs x as [128, nchunks, NK, 512] so ONE DMA delivers a whole
    # projection chunk (all 4 k-tiles).  Order: xq c0, xk c0, xq c1.., then
    # xk c1.., then wvo + xv chunks (v is consumed last).
    xc_tiles = {}

    def xload(tn, j):
        xt = xcp.tile([128, NK, 512], BF, tag=f"xc_{tn}", name=f"x_{tn}_{j}")
        sy.dma_start(xt[:], dram[tn][:, j, :, :])
        xc_tiles[(tn, j)] = xt

    xload("xq", 0)
    xload("xk", 0)
    for j in range(1, len(JQ)):
        xload("xq", j)
    for j in range(1, len(JK)):
        xload("xk", j)
    sy.dma_start(wvo[:], dram["wvo"])
    for j in range(len(JK)):
        xload("xv", j)

    # named projection outputs
    qn = nbuf.tile([128, Tq], BF, tag="qn", name="qn")
    kn = nbuf.tile([128, Tk], BF, tag="kn", name="kn")
    vn = nbuf.tile([128, Tk], FR, tag="vn", name="vn")  # fp32r: feeds transpose
    # lazy per-column LN scales for k/v, transposed to [128, 2*NIK] layout
    # (col = 2*i + h); k's is folded into the exp scale operand, v's into
    # the vaug transpose drain
    rsqk = nbuf.tile([128, 2 * NIK], FP, tag="rsqk", name="rsqk")
    rsqv = nbuf.tile([128, 2 * NIK], FP, tag="rsqv", name="rsqv")
    # vaugT: per (i, h) a [128, 65] block: cols 0..63 = v^T, col 64 = ones
    vaug = nbuf.tile([128, NIK * HPC * 65], BF, tag="vaug", name="vaug")
    vaug3 = vaug[:].rearrange("p (n c) -> p n c", c=65)
    gp.memset(vaug[:], 1.0)
    avn = nbuf.tile([128, Tq], BF, tag="avn", name="avn")

    # PSUM pool stack: ps at the bottom (also hosts rbp/po at the tail),
    # pj on top (released once all projections are emitted), then pav.
    ps_pool = tc.alloc_tile_pool(name="ps", bufs=2, space="PSUM")
    pj = tc.alloc_tile_pool(name="pj", bufs=2, space="PSUM")

    def project_chunk(tn, wname, joff, jw, cscale, bn, gn, bln, dsq_eng, sub=None):
        """Project x cols [joff, joff+jw) and apply per-head LN.  sub=N
        splits the LN chain into N-wide pieces to cut its serial latency
        (used for the chunks gating the first exps).  For k/v (without
        affine) the LN is LAZY: out keeps the centered diff, and only the
        tiny transposed per-column scale 1/sd is computed (folded into the
        exp scale / vaug drain later)."""
        if sub is not None and jw > sub:
            for o in range(joff, joff + jw, sub):
                project_chunk(
                    tn, wname, o, min(sub, joff + jw - o), cscale, bn, gn, bln, dsq_eng
                )
            return
        out = {"xq": qn, "xk": kn, "xv": vn}[tn]
        lazy = tn != "xq" and not use_affine
        j = joff // 512
        so = joff - 512 * j
        pp = pj.tile([128, 512], FP, tag="pp", name=f"pp_{tn}_{joff}")
        for k in range(NK):
            te.matmul(
                pp[:, :jw],
                wT[wname][:, 128 * k : 128 * (k + 1)],
                xc_tiles[(tn, j)][:, k, so : so + jw],
                start=(k == 0),
                stop=(k == NK - 1),
            )
        if lazy:
            # out <- diff (+bc) directly; dsq and the [128,2]-wide var
            # matmuls per tk tile feed the transposed scale tile
            oslc = out[:, joff : joff + jw]
            v.tensor_scalar_add(
                oslc, pp[:, :jw], bias_tiles[bn][:] if use_bias else 0.0
            )
            dsq = sdp.tile([128, 512], BF, tag="dsq", name=f"dsq_{tn}_{j}")
            osq = oslc.bitcast(FP) if tn == "xv" else oslc
            gp.tensor_mul(dsq[:, :jw], osq, osq)
            rtile = rsqk if tn == "xk" else rsqv
            varp = pj.tile([128, 512], FP, tag="aux", bufs=2,
                           name=f"varp_{tn}_{joff}")
            for ii in range(jw // 128):
                i = joff // 128 + ii
                te.matmul(
                    varp[:, 2 * i : 2 * i + 2],
                    dsq[:, 128 * ii : 128 * (ii + 1)],
                    m2vec,
                    start=True,
                    stop=True,
                )
            i0 = joff // 128
            nii = jw // 128
            sdc = sdp.tile([128, 2 * NIK], FP, tag="sdc", name=f"sdc_{tn}_{j}")
            sc.activation(
                sdc[:, 2 * i0 : 2 * (i0 + nii)],
                varp[:, 2 * i0 : 2 * (i0 + nii)],
                AF.Sqrt,
                bias=epst[:],
            )
            v.reciprocal(
                rtile[:, 2 * i0 : 2 * (i0 + nii)], sdc[:, 2 * i0 : 2 * (i0 + nii)]
            )
            return
        if use_bias or dsq_eng is not sc:
            # SBUF copy of diff: two-PSUM-input ops are illegal, so non-ACT
            # squares (and the bias add) go through SBUF
            u = sdp.tile([128, 512], FP, tag="u", name=f"u_{tn}_{j}")
            v.tensor_scalar_add(
                u[:, :jw], pp[:, :jw], bias_tiles[bn][:] if use_bias else 0.0
            )
            diff = u
        else:
            diff = pp
        dsq = sdp.tile([128, 512], BF, tag="dsq", name=f"dsq_{tn}_{j}")
        if dsq_eng is sc:
            sc.activation(dsq[:, :jw], diff[:, :jw], AF.Square)
        else:
            dsq_eng.tensor_mul(dsq[:, :jw], diff[:, :jw], diff[:, :jw])
        pvar = pj.tile([128, 512], FP, tag="aux", bufs=2, name=f"pvar_{tn}_{j}")
        te.matmul(pvar[:, :jw], m2b, dsq[:, :jw], start=True, stop=True)
        sd = sdp.tile([128, 512], FP, tag="sd", name=f"sd_{tn}_{j}")
        sc.activation(sd[:, :jw], pvar[:, :jw], AF.Sqrt, bias=epst[:])
        rsq = sdp.tile([128, 512], FP, tag="rsq", name=f"rsq_{tn}_{j}")
        v.reciprocal(rsq[:, :jw], sd[:, :jw])
        v.scalar_tensor_tensor(
            out[:, joff : joff + jw],
            diff[:, :jw],
            cscale,
            rsq[:, :jw],
            op0=OP.mult,
            op1=OP.mult,
        )
        if use_affine:
            v.tensor_scalar(
                out[:, joff : joff + jw],
                out[:, joff : joff + jw],
                ge_tiles[gn][:],
                ge_tiles[bln][:],
                op0=OP.mult,
                op1=OP.add,
            )

    def vchunk(joff, jw):
        """v projection chunk + transposes of its tk tiles into vaug; the
        lazy LN scale (per-tk 1/sd) is applied in the drain."""
        project_chunk("xv", "wv", joff, jw, 1.0, "bcv", "gev", "bev", gp)
        for ii in range(jw // 128):
            i = joff // 128 + ii
            pt = pj.tile([128, 512], FR, tag="aux", bufs=2, name=f"ptr{i}")[:, 0:128]
            te.transpose(pt[:], vn[:, 128 * i : 128 * (i + 1)], identr)
            src = pt[:].bitcast(FP).rearrange("p (h c) -> p h c", c=DH)
            if use_affine:
                dst = vaug3[:, HPC * i : HPC * i + HPC, 0:DH]
                v.tensor_copy(dst, src)
            else:
                for h in range(HPC):
                    v.tensor_scalar_mul(
                        vaug3[:, HPC * i + h, 0:DH],
                        src[:, h, :],
                        rsqv[:, 2 * i + h : 2 * i + h + 1],
                    )

    # ---- emission: q c0 + k c0 only; remaining q chunks land after the
    # warm-start score units (which need only qn cols 0..512).  256-wide
    # LN sub-chains halve the serial latency to the first exp.
    project_chunk("xq", "wq", *JQ[0], 1.0 / SCALE, "bcq", "geq", "beq", sc)
    project_chunk("xk", "wk", *JK[0], 1.0, "bck", "gek", "bek", sc)

    # ---- attention -----------------------------------------------------
    groups = [list(enumerate(JQ))[g0 : g0 + 2] for g0 in range(0, len(JQ), 2)]
    grp0 = groups[0]

    nts = {}
    av_pending = []
    av_tiles = {}

    def jt_of(joff):
        return 0 if joff < JT[0][1] else 1

    def av_push(i, h, jt):
        for j, (joff, jw) in grp0:
            if jt_of(joff) == jt:
                av_pending.append((i, h, j, joff, jw))

    av_done = []

    def av_emit(limit):
        n = 0
        while av_pending and n < limit:
            i, h, j, joff, jw = av_pending.pop(0)
            jt = jt_of(joff)
            toff = JT[jt][0]
            te.matmul(
                av_tiles[(h, j)][:],
                vaug3[:, HPC * i + h, :],
                nts[(i, h, jt)][:, joff - toff : joff - toff + jw],
                start=(i == 0),
                stop=(i == NIK - 1),
            )
            if i == NIK - 1:
                av_done.append((h, j, joff, jw))
            n += 1

    def unit(i, h, jt):
        hs = slice(DH * h, DH * (h + 1))
        toff, tw = JT[jt]
        ps = ps_pool.tile([128, 1024], FP, tag="ps", name=f"ps{i}_{h}_{jt}")
        for soff, sw in _chunks(tw):
            te.matmul(
                ps[:, soff : soff + sw],
                kn[hs, 128 * i : 128 * (i + 1)],
                qn[hs, toff + soff : toff + soff + sw],
                start=True,
                stop=True,
            )
        nt = ntp.tile([128, 1024], BF, tag="nt", name=f"nt{i}_{h}_{jt}")
        sc.activation(
            nt[:, :tw],
            ps[:, :tw],
            AF.Exp,
            bias=mbias[:, i : i + 1],
            scale=1.0 if use_affine else rsqk[:, 2 * i + h : 2 * i + h + 1],
        )
        nts[(i, h, jt)] = nt
        if pav is not None:
            av_emit(3)

    # mid-stream normalize (phase-0 chunks): DRAM-bounce broadcast of the
    # reciprocal denominator row — no PSUM banks, no ACT time; the DMA
    # round-trip hides under the phase-1 exp stream
    dramp = stack.enter_context(tc.tile_pool(name="dram_scr", bufs=2, space="DRAM"))
    normed = set()

    def norm_pair_mid(h, j, joff, jw):
        avt = av_tiles[(h, j)]
        dnm = dnp.tile([1, 512], FP, tag=f"dnm{h}_{j}", name=f"dnm{h}_{j}")
        v.tensor_copy(dnm[:, :jw], avt[DH : DH + 1, :])
        rdn = dnp.tile([1, 512], FP, tag=f"rdn{h}_{j}", name=f"rdn{h}_{j}")
        v.reciprocal(rdn[:, :jw], dnm[:, :jw])
        dscr = dramp.tile([1, 512], FP, tag="dscr", name=f"dscr{h}_{j}")
        sy.dma_start(dscr[:, :jw], rdn[:, :jw])
        rb = dnp.tile([DH, 512], FP, tag=f"rb{h}_{j}", name=f"rb{h}_{j}")
        sy.dma_start(rb[:, :jw], dscr[:, :jw].partition_broadcast(DH).squeeze(1))
        v.tensor_mul(avn[DH * h : DH * (h + 1), joff : joff + jw],
                     avt[0:DH, :], rb[:, :jw])
        normed.add((h, j))

    def open_pav():
        pj.release()
        p = tc.alloc_tile_pool(name="pav0", bufs=1, space="PSUM")
        for h in range(HPC):
            for j, (joff, jw) in grp0:
                av_tiles[(h, j)] = p.tile(
                    [DH + 1, jw], FP, tag=f"av{h}_{j}", name=f"av{h}_{j}"
                )
        return p

    # warm: jt0 units for i < warm need only qn c0 + kn c0; the remaining
    # qn chunks (smallest first, so the ragged tail is ready earliest)
    # project under them, then the deferred jt1 units catch up.
    pav = None
    warm = min(2, NIK)
    for i in range(warm):
        for h in range(HPC):
            unit(i, h, 0)
            av_push(i, h, 0)
        if i == 0:
            for joff, jw in sorted(JQ[1:], key=lambda c: c[1]):
                project_chunk(
                    "xq", "wq", joff, jw, 1.0 / SCALE, "bcq", "geq", "beq", gp
                )
    for i in range(warm):
        for h in range(HPC):
            for jt in range(1, len(JT)):
                unit(i, h, jt)
                av_push(i, h, jt)

    vi = 0
    ki = 1
    for i in range(warm, NIK):
        if ki < len(JK) and i >= ki + 2:
            project_chunk("xk", "wk", *JK[ki], 1.0, "bck", "gek", "bek", v)
            ki += 1
        if i >= 2 and vi < len(JK):
            vchunk(*JK[vi])
            vi += 1
        for h in range(HPC):
            for jt in range(len(JT)):
                unit(i, h, jt)
                av_push(i, h, jt)
        if pav is None and vi == len(JK) and ki == len(JK):
            pav = open_pav()
    if pav is None:
        while ki < len(JK):
            project_chunk("xk", "wk", *JK[ki], 1.0, "bck", "gek", "bek", v)
            ki += 1
        while vi < len(JK):
            vchunk(*JK[vi])
            vi += 1
        pav = open_pav()
    av_emit(10**9)

    # ---- normalize + out_proj + store ---------------------------------
    # Denominator rows are gathered into one tile per group (row r), one
    # reciprocal, then a per-row PE broadcast (onesP rows keep base
    # partitions aligned).  out_proj PSUM rides the ps tag ring and is
    # DMA'd straight to DRAM with a bf16 cast (gpsimd-initiated).
    outp = stack.enter_context(tc.tile_pool(name="outsb", bufs=2))

    def norm_emit(grp, av_tiles):
        pairs = [(h, j, joff, jw) for j, (joff, jw) in grp for h in range(HPC)
                 if (h, j) not in normed]
        if not pairs:
            return
        # recip rows live at (partition 32*(r%2), col slot 512*(r//2)) so the
        # broadcast matmul operands sit at legal base partitions (0/32)
        rw = 512 * ((len(pairs) + 1) // 2)
        rcp = dnp.tile([128, 1024], FR, tag="rcp", name="rcp")
        rslc = lambda r, jw: rcp[
            32 * (r % 2) : 32 * (r % 2) + 1, 512 * (r // 2) : 512 * (r // 2) + jw
        ]
        avsbs = {}
        for r, (h, j, joff, jw) in enumerate(pairs):
            avt = av_tiles[(h, j)]
            avsb = dnp.tile([DH, 512], FP, tag=f"avsb{r % 4}", name=f"avsb{h}_{j}")
            avsbs[(h, j)] = avsb
            sc.copy(avsb[:, :jw], avt[0:DH, :])
            v.tensor_copy(rslc(r, jw), avt[DH : DH + 1, :])
        with nc.allow_low_precision(reason="fp32r recip rows for PE broadcast"):
            v.reciprocal(rcp[0:33, :rw], rcp[0:33, :rw])
        for r, (h, j, joff, jw) in enumerate(pairs):
            hs = slice(DH * h, DH * (h + 1))
            rbp = ps_pool.tile([128, 512], FP, tag="ps", name=f"rbp{h}_{j}")
            te.matmul(
                rbp[0:DH, :jw],
                onesP[32 * (r % 2) : 32 * (r % 2) + 1, :],
                rslc(r, jw),
                start=True,
                stop=True,
            )
            v.tensor_mul(
                avn[hs, joff : joff + jw], avsbs[(h, j)][:, :jw], rbp[0:DH, :jw]
            )

    def out_emit(grp):
        # per-(j, m) out DMAs: each fires right after its own drain — the
        # tail DMA queue is idle, so pipelining beats batching here
        for j, (joff, jw) in grp:
            ot = outp.tile([128, 4, 512], BF, tag="ot", name=f"ot{j}")
            for m in range(4):
                po = ps_pool.tile([128, 512], FP, tag="ps", name=f"po{j}_{m}")
                te.matmul(
                    po[:, :jw],
                    woT[:, 128 * m : 128 * (m + 1)],
                    avn[:, joff : joff + jw],
                    start=True,
                    stop=True,
                )
                if m == 0:
                    v.tensor_copy(ot[:, m, :jw], po[:, :jw])
                else:
                    sc.copy(ot[:, m, :jw], po[:, :jw])
                sy.dma_start(
                    dram["out"][:, j, m : m + 1, :jw], ot[:, m : m + 1, :jw]
                )

    # per-j tail: j0's normalize+out overlaps the last exps and j1's chain
    for jj in grp0:
        norm_emit([jj], av_tiles)
        out_emit([jj])
    pav.release()
    if len(groups) > 1:
        grp = groups[1]
        pav1 = tc.alloc_tile_pool(name="pav1", bufs=1, space="PSUM")
        av1 = {
            (h, j): pav1.tile([DH + 1, jw], FP, tag=f"av{h}_{j}", name=f"avx{h}_{j}")
            for j, (joff, jw) in grp
            for h in range(HPC)
        }
        for i in range(NIK):
            for h in range(HPC):
                for j, (joff, jw) in grp:
                    jt = jt_of(joff)
                    toff = JT[jt][0]
                    te.matmul(
                        av1[(h, j)][:],
                        vaug3[:, HPC * i + h, :],
                        nts[(i, h, jt)][:, joff - toff : joff - toff + jw],
                        start=(i == 0),
                        stop=(i == NIK - 1),
                    )
        norm_emit(grp, av1)
        pav1.release()
        out_emit(grp)
    ps_pool.release()

    for dname in dbg:
        src = {"qn": qn, "kn": kn, "vn": vn, "avn": avn}[dname]
        eng = gp if dname == "vn" else sy
        eng.dma_start(dram["dbg_" + dname][:, : src.shape[1]], src[:])

    stack.close()


_last_dims = (1024, 1024)


def _build(use_bias, use_affine, debug_names="", Tq=None, Tk=None):
    if Tq is None or Tk is None:
        Tq, Tk = _last_dims
    return _build_impl(use_bias, use_affine, debug_names, Tq, Tk)


@functools.lru_cache(maxsize=4)
def _build_impl(use_bias, use_affine, debug_names, Tq, Tk):
    nc = bass.Bass(
        "TRN2", target_bir_lowering=False, debug=False, num_devices=NCORES
    )
    NIK = Tk // 128
    NJQ = (Tq + 511) // 512
    NJK = (Tk + 511) // 512
    dram = {}
    dram["xq"] = nc.dram_tensor("xq", [128, NJQ, NK, 512], BF, kind="ExternalInput").ap()
    dram["xk"] = nc.dram_tensor("xk", [128, NJK, NK, 512], BF, kind="ExternalInput").ap()
    dram["xv"] = nc.dram_tensor("xv", [128, NJK, NK, 512], BF, kind="ExternalInput").ap()
    dram["wqk"] = nc.dram_tensor(
        "wqk", [128, 2 * E + 130], BF, kind="ExternalInput"
    ).ap()
    dram["wvo"] = nc.dram_tensor("wvo", [128, 2 * E], BF, kind="ExternalInput").ap()
    dram["cpackr"] = nc.dram_tensor(
        "cpackr", [128, 128 + DH], FR, kind="ExternalInput"
    ).ap()
    dram["mbias"] = nc.dram_tensor("mbias", [128, NIK], FP, kind="ExternalInput").ap()
    if use_bias:
        for bn in ("bcq", "bck", "bcv"):
            dram[bn] = nc.dram_tensor(bn, [128, 1], FP, kind="ExternalInput").ap()
    if use_affine:
        for gn in ("geq", "beq", "gek", "bek", "gev", "bev"):
            dram[gn] = nc.dram_tensor(gn, [128, 1], FP, kind="ExternalInput").ap()
    dram["out"] = nc.dram_tensor(
        "out", [128, NJQ, 4, 512], BF, kind="ExternalOutput"
    ).ap()
    dbg = frozenset(debug_names.split(",")) - {""} if debug_names else frozenset()
    for dname in dbg:
        w = Tq if dname in ("qn", "avn") else Tk
        dram["dbg_" + dname] = nc.dram_tensor(
            "dbg_" + dname, [128, w], BF, kind="ExternalOutput"
        ).ap()

    flags = {"use_bias": use_bias, "use_affine": use_affine}
    with tile.TileContext(nc) as tc:
        _emit(nc, tc, dram, flags, dbg, Tq, Tk)
    _split_multiwaits(nc)
    return nc


def _pad_up(n, m):
    return max(m, ((n + m - 1) // m) * m)


@functools.lru_cache(maxsize=1)
def _m2_const():
    m2 = np.zeros((128, 128), np.float32)
    m2[:DH, :DH] = 1.0 / DH
    m2[DH:, DH:] = 1.0 / DH
    return m2


def _std(w):
    mu = w.mean(axis=1, keepdims=True)
    var = w.var(axis=1, keepdims=True)
    return (w - mu) / np.sqrt(var + EPS)


_last_results = None


def kernel(**inputs):
    global _last_results, _last_dims
    a = {k: np.asarray(val) for k, val in inputs.items()}
    use_bias = bool(any(np.any(a[bn] != 0) for bn in ("bq", "bk", "bv")))
    use_affine = bool(
        any(np.any(a[gn] != 1) for gn in ("ln_gq", "ln_gk", "ln_gv"))
        or any(np.any(a[bn] != 0) for bn in ("ln_bq", "ln_bk", "ln_bv"))
    )
    debug_names = os.environ.get("KDEBUG", "")

    qm = a["query_mask"].astype(bool)
    km = a["key_mask"].astype(bool)
    kept_q = [np.flatnonzero(qm[b]) for b in range(B)]
    kept_k = [np.flatnonzero(km[b]) for b in range(B)]
    nq = [len(ix) for ix in kept_q]
    nk = [len(ix) for ix in kept_k]
    Tq = _pad_up(max(nq), 128)
    Tk = _pad_up(max(nk), 128)
    NIK = Tk // 128

    _last_dims = (Tq, Tk)
    nc = _build(use_bias, use_affine, debug_names, Tq, Tk)

    m2 = _m2_const()
    i128 = np.eye(128, dtype=np.float32)
    wsn = {wn: _std(a[wn].astype(np.float32)) for wn in ("Wq", "Wk", "Wv", "Wo")}

    NJQ = (Tq + 511) // 512
    NJK = (Tk + 511) // 512

    def xpack(x, n, NJ):
        # [E, n<=512*NJ] -> [128, NJ, NK, 512] with zero padding
        full = np.zeros((E, 512 * NJ), np.float32)
        full[:, :n] = x[:, :n]
        return np.ascontiguousarray(
            full.reshape(NK, 128, NJ, 512).transpose(1, 2, 0, 3)
        )

    in_maps = []
    for c in range(NCORES):
        b, hp = divmod(c, G)
        rs = 128 * hp
        d = {}
        d["xq"] = xpack(a["q"][b][:, kept_q[b]], nq[b], NJQ)
        d["xk"] = xpack(a["k"][b][:, kept_k[b]], nk[b], NJK)
        d["xv"] = xpack(
            (a["v"][b] * a["value_mask"][b][None, :].astype(np.float32))[:, kept_k[b]],
            nk[b],
            NJK,
        )

        wqk = np.zeros((128, 2 * E + 130), np.float32)
        wvo = np.empty((128, 2 * E), np.float32)
        for wi, key in enumerate(("Wq", "Wk", "Wv")):
            blk = (i128 - m2) @ wsn[key][rs : rs + 128]  # fold per-head centering
            dst = wqk if wi < 2 else wvo
            base = (wi % 2) * E if wi < 2 else 0
            for k in range(NK):
                dst[:, base + 128 * k : base + 128 * (k + 1)] = blk[
                    :, 128 * k : 128 * (k + 1)
                ].T
        for m in range(4):
            wvo[:, E + 128 * m : E + 128 * (m + 1)] = wsn["Wo"][
                128 * m : 128 * (m + 1), rs : rs + 128
            ].T
        wqk[:, 2 * E : 2 * E + 128] = m2
        wqk[:DH, 2 * E + 128] = 1.0 / DH
        wqk[DH:, 2 * E + 129] = 1.0 / DH
        d["wqk"] = wqk
        d["wvo"] = wvo

        cpackr = np.zeros((128, 128 + DH), np.float32)
        cpackr[:, 0:128] = i128
        cpackr[:, 128:] = 1.0
        d["cpackr"] = cpackr
        flat = np.zeros(Tk, np.float32)
        flat[nk[b] :] = -BIG
        d["mbias"] = flat.reshape(NIK, 128).T

        if use_bias:
            for bn, key in (("bcq", "bq"), ("bck", "bk"), ("bcv", "bv")):
                bb = a[key].astype(np.float32)[rs : rs + 128]
                d[bn] = (bb - m2 @ bb)[:, None]
        if use_affine:
            # q's cscale stays 1/SCALE; out_q = (LN/SCALE)*g + b/SCALE
            d["geq"] = np.tile(a["ln_gq"], HPC)[:, None]
            d["beq"] = (np.tile(a["ln_bq"], HPC) / SCALE)[:, None]
            d["gek"] = np.tile(a["ln_gk"], HPC)[:, None]
            d["bek"] = np.tile(a["ln_bk"], HPC)[:, None]
            d["gev"] = np.tile(a["ln_gv"], HPC)[:, None]
            d["bev"] = np.tile(a["ln_bv"], HPC)[:, None]
        for k in ("xq", "xk", "xv", "wqk", "wvo"):
            d[k] = np.ascontiguousarray(d[k]).astype(BF_NP)
        for k in ("cpackr", "mbias", "bcq", "bck", "bcv",
                  "geq", "beq", "gek", "bek", "gev", "bev"):
            if k in d:
                d[k] = np.ascontiguousarray(d[k], dtype=np.float32)
        in_maps.append(d)

    res = bass_utils.run_bass_kernel_spmd(
        nc,
        in_maps,
        core_ids=list(range(NCORES)),
        trace=os.environ.get("KTRACE", "0") == "1",
    )
    _last_results = res
    kernel._last_meta = {"Tq": Tq, "Tk": Tk, "nq": nq, "nk": nk,
                         "kept_q": kept_q, "kept_k": kept_k}

    out = np.zeros((B, E, T), np.float32)
    bo = a["bo"].astype(np.float32)
    for b in range(B):
        acc = res.results[G * b]["out"].astype(np.float32)
        for c in range(G * b + 1, G * b + G):
            acc = acc + res.results[c]["out"].astype(np.float32)
        # [128, NJQ, 4, 512] -> [E, 512*NJQ]
        full = acc.transpose(2, 0, 1, 3).reshape(E, 512 * NJQ)
        out[b][:, kept_q[b]] = full[:, : nq[b]]
        out[b] += bo[:, None]
    return out


# revision 117
# speedup vs baseline: 1.0054x; 1.0054x over previous
"""nn_MultiHeadAttention_84954453115654 — Trainium2 Bass kernel, 8 NeuronCores.

Sharding: data-parallel over batch (2) x head-pair-parallel (4 groups of 2
heads).  Core c handles batch b = c//4 and embed rows [128*(c%4), +128)
(= heads 2*(c%4), 2*(c%4)+1).  Host sums the 4 out_proj partials per batch
and adds bo (the row-parallel all-reduce, done at gather time).

Key idea: the masks are inputs, so the host COMPACTS the time axis before
launch.  Only unmasked query columns (Tq ~ half of 2048) and unmasked key
columns (Tk ~ half) are shipped/computed; value_mask is pre-applied to the
compacted xv.  Masked query columns of the output are exactly bo (reference:
softmax scores row zeroed -> av col 0 -> out col = bias), so the host
scatters computed columns back and fills the rest with bo.  This halves DMA
and PE work and quarters the softmax-exp ACT work vs. dense.

Weight prep happens on host (weights are inputs): weight-standardize, fold
the per-head centering (I - M2) into Wq/Wk/Wv so the projection matmul
directly yields diff = p - mean_head(p), transpose into lhsT layout, bf16.

Per-core device program:
  - q/k/v projections (bf16 matmuls, K-chained over 4 tiles of 128) with
    fused per-head LayerNorm: var via block-mean matmul of diff^2, then
    out = (diff * cscale) * reciprocal(sqrt(var + eps)) (1/SCALE folded
    into q's cscale)
  - scores transposed per (tk-tile i, head, tq-tile): S^T[tk, tq] =
    kn_i^T @ qn chunk; softmax exp on ACT with a per-partition bias (-80
    on compact-pad keys, else 0); no row-max subtraction (post-LN scores
    are O(0.1) so exp cannot overflow)
  - av[65, jw] += vaugT_i @ exp accumulated over i in PSUM; vaug carries a
    ones column so row 64 is the softmax denominator; av matmuls are
    spread between exp units to keep the ACT stream dense
  - lazy LN for k/v: kn/vn keep the centered diff; the per-column 1/sd
    (computed pre-transposed via tiny dsq^T @ m2vec matmuls) folds into
    the exp scale operand (k) and the vaug transpose drain (v)
  - normalize per tq-chunk: denominator rows batched into one tile, one
    reciprocal, PE-broadcast per row, avn = avsb * bcast; out_proj
    partial woT[:, m] @ avn drains bf16 and DMAs per (chunk, m) so the
    first chunk's output overlaps the last exps
"""

import os
import sys
import contextlib
import functools

for _p in ("/root/.axon_site/_ro/trn_rl_repo", "/opt/trn_rl_repo"):
    if os.path.isdir(_p) and _p not in sys.path:
        sys.path.append(_p)

import numpy as np
import ml_dtypes

import concourse.bass as bass
import concourse.mybir as mybir
import concourse.tile as tile
from concourse import bass_utils

B, E, T, H = 2, 512, 2048, 8
DH = E // H            # 64
HPC = 2                # heads per core
G = 4                  # cores per batch group
NCORES = 8
NK = E // 128          # 4 contraction tiles for the projections
EPS = 1e-5
SCALE = float(E // H ** 0.5)   # 181.0
BIG = 80.0
FP = mybir.dt.float32
FR = mybir.dt.float32r
BF = mybir.dt.bfloat16
AF = mybir.ActivationFunctionType
OP = mybir.AluOpType
BF_NP = np.dtype(ml_dtypes.bfloat16)


def _split_multiwaits(nc):
    """Split multi-wait instructions (Tile's tail drain) into single-wait
    EventSemaphore chains; this container's walrus encodes only one sync
    wait per instruction."""
    import bass_rust

    n_new = 0
    for f in nc.m.functions:
        for bb in f.blocks:
            out = []
            changed = False
            for ins in bb.instructions:
                si = ins.sync_info
                if si is not None and si.on_wait is not None and len(si.on_wait) > 1:
                    waits = list(si.on_wait)
                    for w in waits[:-1]:
                        ev = bass_rust.InstEventSemaphore(
                            name=f"MWFIX-{n_new}", ins=[], outs=[]
                        )
                        n_new += 1
                        ev.engine = ins.engine
                        ev.sync_info = bass_rust.SyncInfo(on_wait=[w], on_update=[])
                        out.append(ev)
                    ins.sync_info = bass_rust.SyncInfo(
                        on_wait=[waits[-1]], on_update=list(si.on_update or [])
                    )
                    changed = True
                out.append(ins)
            if changed:
                bb.instructions = out
    return n_new


def _chunks(total, step=512):
    out = []
    off = 0
    while off < total:
        out.append((off, min(step, total - off)))
        off += step
    return out


def _jt_split(Tq):
    """scores/exp tile widths: one tile if it fits 2 PSUM banks, else
    (512, rest) so the first exp only needs the first qn chunk."""
    if Tq <= 1024:
        return [(0, Tq)]
    assert Tq <= 1536, f"Tq={Tq} too large for the 2-tile scores layout"
    return [(0, 512), (512, Tq - 512)]


def _emit(nc, tc, dram, flags, dbg, Tq, Tk):
    v = nc.vector
    sc = nc.scalar
    te = nc.tensor
    gp = nc.gpsimd
    sy = nc.sync

    NIK = Tk // 128          # tk tiles
    JQ = _chunks(Tq)         # tq chunks (512-wide, last may be ragged)
    JK = _chunks(Tk)
    JT = _jt_split(Tq)
    use_bias = flags["use_bias"]
    use_affine = flags["use_affine"]

    stack = contextlib.ExitStack()

    consts = stack.enter_context(tc.tile_pool(name="consts", bufs=1))
    xcp = stack.enter_context(tc.tile_pool(name="xc", bufs=2))
    nbuf = stack.enter_context(tc.tile_pool(name="named", bufs=1))
    ntp = stack.enter_context(
        tc.tile_pool(name="nt", bufs=(Tk // 128) * HPC * len(_jt_split(Tq)) + 2)
    )
    sdp = stack.enter_context(tc.tile_pool(name="sd", bufs=3))
    dnp = stack.enter_context(tc.tile_pool(name="dn", bufs=1))

    # ---- packed constant loads -----------------------------------------
    # wqk: [wq | wk | m2b] bf16 (needed before the first exp);
    # wvo: [wv | woT] bf16 (queued after the early x chunks);
    # cpackr: [identr | onesP] fp32r; mbias separate fp32.
    cpackr = consts.tile([128, 128 + DH], FR, tag="cpackr")
    sy.dma_start(cpackr[:], dram["cpackr"])
    identr = cpackr[:, 0:128]
    onesP = cpackr[:, 128 : 128 + DH]  # all-ones rows (base-aligned lhsT)
    mbias = consts.tile([128, NIK], FP, tag="mbias")
    sy.dma_start(mbias[:], dram["mbias"])
    wqk = consts.tile([128, 2 * E + 130], BF, tag="wqk")
    sy.dma_start(wqk[:], dram["wqk"])
    wvo = consts.tile([128, 2 * E], BF, tag="wvo")
    wT = {"wq": wqk[:, 0:E], "wk": wqk[:, E : 2 * E], "wv": wvo[:, 0:E]}
    m2b = wqk[:, 2 * E : 2 * E + 128]
    m2vec = wqk[:, 2 * E + 128 : 2 * E + 130]  # per-head 1/64 column pair
    woT = wvo[:, E : 2 * E]
    epst = consts.tile([128, 1], FP, tag="eps")
    v.memset(epst[:], EPS)

    bias_tiles = {}
    if use_bias:
        for bn in ("bcq", "bck", "bcv"):
            bt = consts.tile([128, 1], FP, tag=bn, name=bn)
            sy.dma_start(bt[:], dram[bn])
            bias_tiles[bn] = bt
    ge_tiles = {}
    if use_affine:
        for gn in ("geq", "beq", "gek", "bek", "gev", "bev"):
            gt = consts.tile([128, 1], FP, tag=gn, name=gn)
            sy.dma_start(gt[:], dram[gn])
            ge_tiles[gn] = gt

    # ---- x loads -------------------------------------------------------
    # Host packs x as [128, nchunks, NK, 512] so ONE DMA delivers a whole
    # projection chunk (all 4 k-tiles).  Order: xq c0, xk c0, xq c1.., then
    # xk c1.., then wvo + xv chunks (v is consumed last).
    xc_tiles = {}

    def xload(tn, j):
        xt = xcp.tile([128, NK, 512], BF, tag=f"xc_{tn}", name=f"x_{tn}_{j}")
        sy.dma_start(xt[:], dram[tn][:, j, :, :])
        xc_tiles[(tn, j)] = xt

    xload("xq", 0)
    xload("xk", 0)
    for j in range(1, len(JQ)):
        xload("xq", j)
    for j in range(1, len(JK)):
        xload("xk", j)
    sy.dma_start(wvo[:], dram["wvo"])
    for j in range(len(JK)):
        xload("xv", j)

    # named projection outputs
    qn = nbuf.tile([128, Tq], BF, tag="qn", name="qn")
    kn = nbuf.tile([128, Tk], BF, tag="kn", name="kn")
    vn = nbuf.tile([128, Tk], FR, tag="vn", name="vn")  # fp32r: feeds transpose
    # lazy per-column LN scales for k/v, transposed to [128, 2*NIK] layout
    # (col = 2*i + h); k's is folded into the exp scale operand, v's into
    # the vaug transpose drain
    rsqk = nbuf.tile([128, 2 * NIK], FP, tag="rsqk", name="rsqk")
    rsqv = nbuf.tile([128, 2 * NIK], FP, tag="rsqv", name="rsqv")
    # vaugT: per (i, h) a [128, 65] block: cols 0..63 = v^T, col 64 = ones
    vaug = nbuf.tile([128, NIK * HPC * 65], BF, tag="vaug", name="vaug")
    vaug3 = vaug[:].rearrange("p (n c) -> p n c", c=65)
    gp.memset(vaug[:], 1.0)
    avn = nbuf.tile([128, Tq], BF, tag="avn", name="avn")

    # PSUM pool stack: ps at the bottom (also hosts rbp/po at the tail),
    # pj on top (released once all projections are emitted), then pav.
    ps_pool = tc.alloc_tile_pool(name="ps", bufs=2, space="PSUM")
    pj = tc.alloc_tile_pool(name="pj", bufs=2, space="PSUM")

    def project_chunk(tn, wname, joff, jw, cscale, bn, gn, bln, dsq_eng, sub=None):
        """Project x cols [joff, joff+jw) and apply per-head LN.  sub=N
        splits the LN chain into N-wide pieces to cut its serial latency
        (used for the chunks gating the first exps).  For k/v (without
        affine) the LN is LAZY: out keeps the centered diff, and only the
        tiny transposed per-column scale 1/sd is computed (folded into the
        exp scale / vaug drain later)."""
        if sub is not None and jw > sub:
            for o in range(joff, joff + jw, sub):
                project_chunk(
                    tn, wname, o, min(sub, joff + jw - o), cscale, bn, gn, bln, dsq_eng
                )
            return
        out = {"xq": qn, "xk": kn, "xv": vn}[tn]
        lazy = tn != "xq" and not use_affine
        j = joff // 512
        so = joff - 512 * j
        pp = pj.tile([128, 512], FP, tag="pp", name=f"pp_{tn}_{joff}")
        for k in range(NK):
            te.matmul(
                pp[:, :jw],
                wT[wname][:, 128 * k : 128 * (k + 1)],
                xc_tiles[(tn, j)][:, k, so : so + jw],
                start=(k == 0),
                stop=(k == NK - 1),
            )
        if lazy:
            # out <- diff (+bc) directly; dsq and the [128,2]-wide var
            # matmuls per tk tile feed the transposed scale tile
            oslc = out[:, joff : joff + jw]
            v.tensor_scalar_add(
                oslc, pp[:, :jw], bias_tiles[bn][:] if use_bias else 0.0
            )
            dsq = sdp.tile([128, 512], BF, tag="dsq", name=f"dsq_{tn}_{j}")
            osq = oslc.bitcast(FP) if tn == "xv" else oslc
            gp.tensor_mul(dsq[:, :jw], osq, osq)
            rtile = rsqk if tn == "xk" else rsqv
            varp = pj.tile([128, 512], FP, tag="aux", bufs=2,
                           name=f"varp_{tn}_{joff}")
            for ii in range(jw // 128):
                i = joff // 128 + ii
                te.matmul(
                    varp[:, 2 * i : 2 * i + 2],
                    dsq[:, 128 * ii : 128 * (ii + 1)],
                    m2vec,
                    start=True,
                    stop=True,
                )
            i0 = joff // 128
            nii = jw // 128
            sdc = sdp.tile([128, 2 * NIK], FP, tag="sdc", name=f"sdc_{tn}_{j}")
            sc.activation(
                sdc[:, 2 * i0 : 2 * (i0 + nii)],
                varp[:, 2 * i0 : 2 * (i0 + nii)],
                AF.Sqrt,
                bias=epst[:],
            )
            v.reciprocal(
                rtile[:, 2 * i0 : 2 * (i0 + nii)], sdc[:, 2 * i0 : 2 * (i0 + nii)]
            )
            return
        if use_bias or dsq_eng is not sc:
            # SBUF copy of diff: two-PSUM-input ops are illegal, so non-ACT
            # squares (and the bias add) go through SBUF
            u = sdp.tile([128, 512], FP, tag="u", name=f"u_{tn}_{j}")
            v.tensor_scalar_add(
                u[:, :jw], pp[:, :jw], bias_tiles[bn][:] if use_bias else 0.0
            )
            diff = u
        else:
            diff = pp
        dsq = sdp.tile([128, 512], BF, tag="dsq", name=f"dsq_{tn}_{j}")
        if dsq_eng is sc:
            sc.activation(dsq[:, :jw], diff[:, :jw], AF.Square)
        else:
            dsq_eng.tensor_mul(dsq[:, :jw], diff[:, :jw], diff[:, :jw])
        pvar = pj.tile([128, 512], FP, tag="aux", bufs=2, name=f"pvar_{tn}_{j}")
        te.matmul(pvar[:, :jw], m2b, dsq[:, :jw], start=True, stop=True)
        sd = sdp.tile([128, 512], FP, tag="sd", name=f"sd_{tn}_{j}")
        sc.activation(sd[:, :jw], pvar[:, :jw], AF.Sqrt, bias=epst[:])
        rsq = sdp.tile([128, 512], FP, tag="rsq", name=f"rsq_{tn}_{j}")
        v.reciprocal(rsq[:, :jw], sd[:, :jw])
        v.scalar_tensor_tensor(
            out[:, joff : joff + jw],
            diff[:, :jw],
            cscale,
            rsq[:, :jw],
            op0=OP.mult,
            op1=OP.mult,
        )
        if use_affine:
            v.tensor_scalar(
                out[:, joff : joff + jw],
                out[:, joff : joff + jw],
                ge_tiles[gn][:],
                ge_tiles[bln][:],
                op0=OP.mult,
                op1=OP.add,
            )

    def vchunk(joff, jw):
        """v projection chunk + transposes of its tk tiles into vaug; the
        lazy LN scale (per-tk 1/sd) is applied in the drain."""
        project_chunk("xv", "wv", joff, jw, 1.0, "bcv", "gev", "bev", gp)
        for ii in range(jw // 128):
            i = joff // 128 + ii
            pt = pj.tile([128, 512], FR, tag="aux", bufs=2, name=f"ptr{i}")[:, 0:128]
            te.transpose(pt[:], vn[:, 128 * i : 128 * (i + 1)], identr)
            src = pt[:].bitcast(FP).rearrange("p (h c) -> p h c", c=DH)
            if use_affine:
                dst = vaug3[:, HPC * i : HPC * i + HPC, 0:DH]
                v.tensor_copy(dst, src)
            else:
                for h in range(HPC):
                    v.tensor_scalar_mul(
                        vaug3[:, HPC * i + h, 0:DH],
                        src[:, h, :],
                        rsqv[:, 2 * i + h : 2 * i + h + 1],
                    )

    # ---- emission: q c0 + k c0 only; remaining q chunks land after the
    # warm-start score units (which need only qn cols 0..512).  256-wide
    # LN sub-chains halve the serial latency to the first exp.
    project_chunk("xq", "wq", *JQ[0], 1.0 / SCALE, "bcq", "geq", "beq", sc)
    project_chunk("xk", "wk", *JK[0], 1.0, "bck", "gek", "bek", sc)

    # ---- attention -----------------------------------------------------
    groups = [list(enumerate(JQ))[g0 : g0 + 2] for g0 in range(0, len(JQ), 2)]
    grp0 = groups[0]

    nts = {}
    av_pending = []
    av_tiles = {}

    def jt_of(joff):
        return 0 if joff < JT[0][1] else 1

    def av_push(i, h, jt):
        for j, (joff, jw) in grp0:
            if jt_of(joff) == jt:
                av_pending.append((i, h, j, joff, jw))

    av_done = []

    def av_emit(limit):
        n = 0
        while av_pending and n < limit:
            i, h, j, joff, jw = av_pending.pop(0)
            jt = jt_of(joff)
            toff = JT[jt][0]
            te.matmul(
                av_tiles[(h, j)][:],
                vaug3[:, HPC * i + h, :],
                nts[(i, h, jt)][:, joff - toff : joff - toff + jw],
                start=(i == 0),
                stop=(i == NIK - 1),
            )
            if i == NIK - 1:
                av_done.append((h, j, joff, jw))
            n += 1

    def unit(i, h, jt):
        hs = slice(DH * h, DH * (h + 1))
        toff, tw = JT[jt]
        ps = ps_pool.tile([128, 1024], FP, tag="ps", name=f"ps{i}_{h}_{jt}")
        for soff, sw in _chunks(tw):
            te.matmul(
                ps[:, soff : soff + sw],
                kn[hs, 128 * i : 128 * (i + 1)],
                qn[hs, toff + soff : toff + soff + sw],
                start=True,
                stop=True,
            )
        nt = ntp.tile([128, 1024], BF, tag="nt", name=f"nt{i}_{h}_{jt}")
        sc.activation(
            nt[:, :tw],
            ps[:, :tw],
            AF.Exp,
            bias=mbias[:, i : i + 1],
            scale=1.0 if use_affine else rsqk[:, 2 * i + h : 2 * i + h + 1],
        )
        nts[(i, h, jt)] = nt
        if pav is not None:
            av_emit(4)

    # mid-stream normalize (phase-0 chunks): DRAM-bounce broadcast of the
    # reciprocal denominator row — no PSUM banks, no ACT time; the DMA
    # round-trip hides under the phase-1 exp stream
    dramp = stack.enter_context(tc.tile_pool(name="dram_scr", bufs=2, space="DRAM"))
    normed = set()

    def norm_pair_mid(h, j, joff, jw):
        avt = av_tiles[(h, j)]
        dnm = dnp.tile([1, 512], FP, tag=f"dnm{h}_{j}", name=f"dnm{h}_{j}")
        v.tensor_copy(dnm[:, :jw], avt[DH : DH + 1, :])
        rdn = dnp.tile([1, 512], FP, tag=f"rdn{h}_{j}", name=f"rdn{h}_{j}")
        v.reciprocal(rdn[:, :jw], dnm[:, :jw])
        dscr = dramp.tile([1, 512], FP, tag="dscr", name=f"dscr{h}_{j}")
        sy.dma_start(dscr[:, :jw], rdn[:, :jw])
        rb = dnp.tile([DH, 512], FP, tag=f"rb{h}_{j}", name=f"rb{h}_{j}")
        sy.dma_start(rb[:, :jw], dscr[:, :jw].partition_broadcast(DH).squeeze(1))
        v.tensor_mul(avn[DH * h : DH * (h + 1), joff : joff + jw],
                     avt[0:DH, :], rb[:, :jw])
        normed.add((h, j))

    def open_pav():
        pj.release()
        p = tc.alloc_tile_pool(name="pav0", bufs=1, space="PSUM")
        for h in range(HPC):
            for j, (joff, jw) in grp0:
                av_tiles[(h, j)] = p.tile(
                    [DH + 1, jw], FP, tag=f"av{h}_{j}", name=f"av{h}_{j}"
                )
        return p

    # warm: jt0 units for i < warm need only qn c0 + kn c0; the remaining
    # qn chunks (smallest first, so the ragged tail is ready earliest)
    # project under them, then the deferred jt1 units catch up.
    pav = None
    warm = min(2, NIK)
    for i in range(warm):
        for h in range(HPC):
            unit(i, h, 0)
            av_push(i, h, 0)
        if i == 0:
            for joff, jw in sorted(JQ[1:], key=lambda c: c[1]):
                project_chunk(
                    "xq", "wq", joff, jw, 1.0 / SCALE, "bcq", "geq", "beq", gp
                )
    for i in range(warm):
        for h in range(HPC):
            for jt in range(1, len(JT)):
                unit(i, h, jt)
                av_push(i, h, jt)

    vi = 0
    ki = 1
    for i in range(warm, NIK):
        if ki < len(JK) and i >= ki + 2:
            project_chunk("xk", "wk", *JK[ki], 1.0, "bck", "gek", "bek", v)
            ki += 1
        if i >= 2 and vi < len(JK):
            vchunk(*JK[vi])
            vi += 1
        for h in range(HPC):
            for jt in range(len(JT)):
                unit(i, h, jt)
                av_push(i, h, jt)
        if pav is None and vi == len(JK) and ki == len(JK):
            pav = open_pav()
    if pav is None:
        while ki < len(JK):
            project_chunk("xk", "wk", *JK[ki], 1.0, "bck", "gek", "bek", v)
            ki += 1
        while vi < len(JK):
            vchunk(*JK[vi])
            vi += 1
        pav = open_pav()
    av_emit(10**9)

    # ---- normalize + out_proj + store ---------------------------------
    # Denominator rows are gathered into one tile per group (row r), one
    # reciprocal, then a per-row PE broadcast (onesP rows keep base
    # partitions aligned).  out_proj PSUM rides the ps tag ring and is
    # DMA'd straight to DRAM with a bf16 cast (gpsimd-initiated).
    outp = stack.enter_context(tc.tile_pool(name="outsb", bufs=2))

    def norm_emit(grp, av_tiles):
        pairs = [(h, j, joff, jw) for j, (joff, jw) in grp for h in range(HPC)
                 if (h, j) not in normed]
        if not pairs:
            return
        # recip rows live at (partition 32*(r%2), col slot 512*(r//2)) so the
        # broadcast matmul operands sit at legal base partitions (0/32)
        rw = 512 * ((len(pairs) + 1) // 2)
        rcp = dnp.tile([128, 1024], FR, tag="rcp", name="rcp")
        rslc = lambda r, jw: rcp[
            32 * (r % 2) : 32 * (r % 2) + 1, 512 * (r // 2) : 512 * (r // 2) + jw
        ]
        avsbs = {}
        for r, (h, j, joff, jw) in enumerate(pairs):
            avt = av_tiles[(h, j)]
            avsb = dnp.tile([DH, 512], FP, tag=f"avsb{r % 4}", name=f"avsb{h}_{j}")
            avsbs[(h, j)] = avsb
            sc.copy(avsb[:, :jw], avt[0:DH, :])
            v.tensor_copy(rslc(r, jw), avt[DH : DH + 1, :])
        with nc.allow_low_precision(reason="fp32r recip rows for PE broadcast"):
            v.reciprocal(rcp[0:33, :rw], rcp[0:33, :rw])
        for r, (h, j, joff, jw) in enumerate(pairs):
            hs = slice(DH * h, DH * (h + 1))
            rbp = ps_pool.tile([128, 512], FP, tag="ps", name=f"rbp{h}_{j}")
            te.matmul(
                rbp[0:DH, :jw],
                onesP[32 * (r % 2) : 32 * (r % 2) + 1, :],
                rslc(r, jw),
                start=True,
                stop=True,
            )
            v.tensor_mul(
                avn[hs, joff : joff + jw], avsbs[(h, j)][:, :jw], rbp[0:DH, :jw]
            )

    def out_emit(grp):
        # per-(j, m) out DMAs: each fires right after its own drain — the
        # tail DMA queue is idle, so pipelining beats batching here
        for j, (joff, jw) in grp:
            ot = outp.tile([128, 4, 512], BF, tag="ot", name=f"ot{j}")
            for m in range(4):
                po = ps_pool.tile([128, 512], FP, tag="ps", name=f"po{j}_{m}")
                te.matmul(
                    po[:, :jw],
                    woT[:, 128 * m : 128 * (m + 1)],
                    avn[:, joff : joff + jw],
                    start=True,
                    stop=True,
                )
                if m == 0:
                    v.tensor_copy(ot[:, m, :jw], po[:, :jw])
                else:
                    sc.copy(ot[:, m, :jw], po[:, :jw])
                if jw == 512:
                    sy.dma_start(
                        dram["out"][:, j, m : m + 1, :], ot[:, m : m + 1, :]
                    )
            if jw < 512:
                # ragged chunk: one merged DMA, avoids 3 extra HWDGE launches
                sy.dma_start(dram["out"][:, j, :, :jw], ot[:, :, :jw])

    # per-j tail: j0's normalize+out overlaps the last exps and j1's chain
    for jj in grp0:
        norm_emit([jj], av_tiles)
        out_emit([jj])
    pav.release()
    if len(groups) > 1:
        grp = groups[1]
        pav1 = tc.alloc_tile_pool(name="pav1", bufs=1, space="PSUM")
        av1 = {
            (h, j): pav1.tile([DH + 1, jw], FP, tag=f"av{h}_{j}", name=f"avx{h}_{j}")
            for j, (joff, jw) in grp
            for h in range(HPC)
        }
        for i in range(NIK):
            for h in range(HPC):
                for j, (joff, jw) in grp:
                    jt = jt_of(joff)
                    toff = JT[jt][0]
                    te.matmul(
                        av1[(h, j)][:],
                        vaug3[:, HPC * i + h, :],
                        nts[(i, h, jt)][:, joff - toff : joff - toff + jw],
                        start=(i == 0),
                        stop=(i == NIK - 1),
                    )
        norm_emit(grp, av1)
        pav1.release()
        out_emit(grp)
    ps_pool.release()

    for dname in dbg:
        src = {"qn": qn, "kn": kn, "vn": vn, "avn": avn}[dname]
        eng = gp if dname == "vn" else sy
        eng.dma_start(dram["dbg_" + dname][:, : src.shape[1]], src[:])

    stack.close()


_last_dims = (1024, 1024)


def _build(use_bias, use_affine, debug_names="", Tq=None, Tk=None):
    if Tq is None or Tk is None:
        Tq, Tk = _last_dims
    return _build_impl(use_bias, use_affine, debug_names, Tq, Tk)


@functools.lru_cache(maxsize=4)
def _build_impl(use_bias, use_affine, debug_names, Tq, Tk):
    nc = bass.Bass(
        "TRN2", target_bir_lowering=False, debug=False, num_devices=NCORES
    )
    NIK = Tk // 128
    NJQ = (Tq + 511) // 512
    NJK = (Tk + 511) // 512
    dram = {}
    dram["xq"] = nc.dram_tensor("xq", [128, NJQ, NK, 512], BF, kind="ExternalInput").ap()
    dram["xk"] = nc.dram_tensor("xk", [128, NJK, NK, 512], BF, kind="ExternalInput").ap()
    dram["xv"] = nc.dram_tensor("xv", [128, NJK, NK, 512], BF, kind="ExternalInput").ap()
    dram["wqk"] = nc.dram_tensor(
        "wqk", [128, 2 * E + 130], BF, kind="ExternalInput"
    ).ap()
    dram["wvo"] = nc.dram_tensor("wvo", [128, 2 * E], BF, kind="ExternalInput").ap()
    dram["cpackr"] = nc.dram_tensor(
        "cpackr", [128, 128 + DH], FR, kind="ExternalInput"
    ).ap()
    dram["mbias"] = nc.dram_tensor("mbias", [128, NIK], FP, kind="ExternalInput").ap()
    if use_bias:
        for bn in ("bcq", "bck", "bcv"):
            dram[bn] = nc.dram_tensor(bn, [128, 1], FP, kind="ExternalInput").ap()
    if use_affine:
        for gn in ("geq", "beq", "gek", "bek", "gev", "bev"):
            dram[gn] = nc.dram_tensor(gn, [128, 1], FP, kind="ExternalInput").ap()
    dram["out"] = nc.dram_tensor(
        "out", [128, NJQ, 4, 512], BF, kind="ExternalOutput"
    ).ap()
    dbg = frozenset(debug_names.split(",")) - {""} if debug_names else frozenset()
    for dname in dbg:
        w = Tq if dname in ("qn", "avn") else Tk
        dram["dbg_" + dname] = nc.dram_tensor(
            "dbg_" + dname, [128, w], BF, kind="ExternalOutput"
        ).ap()

    flags = {"use_bias": use_bias, "use_affine": use_affine}
    with tile.TileContext(nc) as tc:
        _emit(nc, tc, dram, flags, dbg, Tq, Tk)
    _split_multiwaits(nc)
    return nc


def _pad_up(n, m):
    return max(m, ((n + m - 1) // m) * m)


@functools.lru_cache(maxsize=1)
def _m2_const():
    m2 = np.zeros((128, 128), np.float32)
    m2[:DH, :DH] = 1.0 / DH
    m2[DH:, DH:] = 1.0 / DH
    return m2


def _std(w):
    mu = w.mean(axis=1, keepdims=True)
    var = w.var(axis=1, keepdims=True)
    return (w - mu) / np.sqrt(var + EPS)


_last_results = None


def kernel(**inputs):
    global _last_results, _last_dims
    a = {k: np.asarray(val) for k, val in inputs.items()}
    use_bias = bool(any(np.any(a[bn] != 0) for bn in ("bq", "bk", "bv")))
    use_affine = bool(
        any(np.any(a[gn] != 1) for gn in ("ln_gq", "ln_gk", "ln_gv"))
        or any(np.any(a[bn] != 0) for bn in ("ln_bq", "ln_bk", "ln_bv"))
    )
    debug_names = os.environ.get("KDEBUG", "")

    qm = a["query_mask"].astype(bool)
    km = a["key_mask"].astype(bool)
    kept_q = [np.flatnonzero(qm[b]) for b in range(B)]
    kept_k = [np.flatnonzero(km[b]) for b in range(B)]
    nq = [len(ix) for ix in kept_q]
    nk = [len(ix) for ix in kept_k]
    Tq = _pad_up(max(nq), 128)
    Tk = _pad_up(max(nk), 128)
    NIK = Tk // 128

    _last_dims = (Tq, Tk)
    nc = _build(use_bias, use_affine, debug_names, Tq, Tk)

    m2 = _m2_const()
    i128 = np.eye(128, dtype=np.float32)
    wsn = {wn: _std(a[wn].astype(np.float32)) for wn in ("Wq", "Wk", "Wv", "Wo")}

    NJQ = (Tq + 511) // 512
    NJK = (Tk + 511) // 512

    def xpack(x, n, NJ):
        # [E, n<=512*NJ] -> [128, NJ, NK, 512] with zero padding
        full = np.zeros((E, 512 * NJ), np.float32)
        full[:, :n] = x[:, :n]
        return np.ascontiguousarray(
            full.reshape(NK, 128, NJ, 512).transpose(1, 2, 0, 3)
        )

    in_maps = []
    for c in range(NCORES):
        b, hp = divmod(c, G)
        rs = 128 * hp
        d = {}
        d["xq"] = xpack(a["q"][b][:, kept_q[b]], nq[b], NJQ)
        d["xk"] = xpack(a["k"][b][:, kept_k[b]], nk[b], NJK)
        d["xv"] = xpack(
            (a["v"][b] * a["value_mask"][b][None, :].astype(np.float32))[:, kept_k[b]],
            nk[b],
            NJK,
        )

        wqk = np.zeros((128, 2 * E + 130), np.float32)
        wvo = np.empty((128, 2 * E), np.float32)
        for wi, key in enumerate(("Wq", "Wk", "Wv")):
            blk = (i128 - m2) @ wsn[key][rs : rs + 128]  # fold per-head centering
            dst = wqk if wi < 2 else wvo
            base = (wi % 2) * E if wi < 2 else 0
            for k in range(NK):
                dst[:, base + 128 * k : base + 128 * (k + 1)] = blk[
                    :, 128 * k : 128 * (k + 1)
                ].T
        for m in range(4):
            wvo[:, E + 128 * m : E + 128 * (m + 1)] = wsn["Wo"][
                128 * m : 128 * (m + 1), rs : rs + 128
            ].T
        wqk[:, 2 * E : 2 * E + 128] = m2
        wqk[:DH, 2 * E + 128] = 1.0 / DH
        wqk[DH:, 2 * E + 129] = 1.0 / DH
        d["wqk"] = wqk
        d["wvo"] = wvo

        cpackr = np.zeros((128, 128 + DH), np.float32)
        cpackr[:, 0:128] = i128
        cpackr[:, 128:] = 1.0
        d["cpackr"] = cpackr
        flat = np.zeros(Tk, np.float32)
        flat[nk[b] :] = -BIG
        d["mbias"] = flat.reshape(NIK, 128).T

        if use_bias:
            for bn, key in (("bcq", "bq"), ("bck", "bk"), ("bcv", "bv")):
                bb = a[key].astype(np.float32)[rs : rs + 128]
                d[bn] = (bb - m2 @ bb)[:, None]
        if use_affine:
            # q's cscale stays 1/SCALE; out_q = (LN/SCALE)*g + b/SCALE
            d["geq"] = np.tile(a["ln_gq"], HPC)[:, None]
            d["beq"] = (np.tile(a["ln_bq"], HPC) / SCALE)[:, None]
            d["gek"] = np.tile(a["ln_gk"], HPC)[:, None]
            d["bek"] = np.tile(a["ln_bk"], HPC)[:, None]
            d["gev"] = np.tile(a["ln_gv"], HPC)[:, None]
            d["bev"] = np.tile(a["ln_bv"], HPC)[:, None]
        for k in ("xq", "xk", "xv", "wqk", "wvo"):
            d[k] = np.ascontiguousarray(d[k]).astype(BF_NP)
        for k in ("cpackr", "mbias", "bcq", "bck", "bcv",
                  "geq", "beq", "gek", "bek", "gev", "bev"):
            if k in d:
                d[k] = np.ascontiguousarray(d[k], dtype=np.float32)
        in_maps.append(d)

    res = bass_utils.run_bass_kernel_spmd(
        nc,
        in_maps,
        core_ids=list(range(NCORES)),
        trace=os.environ.get("KTRACE", "0") == "1",
    )
    _last_results = res
    kernel._last_meta = {"Tq": Tq, "Tk": Tk, "nq": nq, "nk": nk,
                         "kept_q": kept_q, "kept_k": kept_k}

    out = np.zeros((B, E, T), np.float32)
    bo = a["bo"].astype(np.float32)
    for b in range(B):
        acc = res.results[G * b]["out"].astype(np.float32)
        for c in range(G * b + 1, G * b + G):
            acc = acc + res.results[c]["out"].astype(np.float32)
        # [128, NJQ, 4, 512] -> [E, 512*NJQ]
        full = acc.transpose(2, 0, 1, 3).reshape(E, 512 * NJQ)
        out[b][:, kept_q[b]] = full[:, : nq[b]]
        out[b] += bo[:, None]
    return out
